# revision 23
# baseline (speedup 1.0000x reference)
"""Distributed GQA attention (RoPE, causal) for 8 TRN2 NeuronCores.

Sharding: tensor-parallel over heads (4 Q heads / 1 KV head per core).
Each core computes full-length Q/K/V projections for its heads, RoPE,
flash-style causal attention with the 4 heads packed into N=512 matmuls,
and its partial output projection; partials are summed with a chunked
bf16 ReduceScatter overlapped with the wo phase.

Layouts are feature-major ("transposed"): activations live as [feat, row]
so every matmul contracts over the partition dim with base partition 0.
Softmax runs max-free (scores are O(5) here), with the denominator
produced for free by a ones-column appended to V.
"""

import numpy as np
import ml_dtypes

B, S, D = 2, 2048, 2048
H, HKV, HD = 32, 8, 64
M = 8                 # cores
HL = H // M           # 4 local Q heads
CH = 128              # position chunk
NCH = S // CH         # 16 chunks per sequence
RT = B * S            # 4096 total rows
QF = HL * HD          # 256 local q features
GROUPS = [(0, 8), (8, 16), (16, 24), (24, 30), (30, 32)]  # RS groups in 128-row chunks
NG = len(GROUPS)

bf16 = ml_dtypes.bfloat16

_CACHE = {}
RUN_OPTS = {}          # test harness may set {"trace": True}
LAST_RESULT = [None]   # test harness reads profiling info from here


def _build_nc():
    import concourse.bacc as bacc
    import concourse.mybir as mybir
    from concourse import tile
    def add_dep_helper(*a, **k):
        pass

    F32, BF16 = mybir.dt.float32, mybir.dt.bfloat16
    Exp = mybir.ActivationFunctionType.Exp

    nc = bacc.Bacc("TRN2", target_bir_lowering=False, debug=False, num_devices=M)

    xt_p = nc.declare_dram_parameter("xt", [D, RT], BF16, isOutput=False)
    cos_p = nc.declare_dram_parameter("cos2", [128, RT], BF16, isOutput=False)
    ssin_p = nc.declare_dram_parameter("ssin2", [128, RT], BF16, isOutput=False)
    wq_p = nc.declare_dram_parameter("wqs", [D, QF], BF16, isOutput=False)
    wkv_p = nc.declare_dram_parameter("wkvs", [D, 2 * HD], BF16, isOutput=False)
    wo_p = nc.declare_dram_parameter("wos", [QF, D], BF16, isOutput=False)
    tri_p = nc.declare_dram_parameter("tri4", [128, 512], BF16, isOutput=False)
    id_p = nc.declare_dram_parameter("ident", [128, 128], BF16, isOutput=False)
    out_p = nc.declare_dram_parameter("out", [QF, RT], BF16, isOutput=True)

    with tile.TileContext(nc) as tc:
        with tc.tile_pool(name="dram", bufs=1, space="DRAM") as dram, \
             tc.tile_pool(name="persist", bufs=1) as per, \
             tc.tile_pool(name="xload", bufs=1) as xload, \
             tc.tile_pool(name="work", bufs=2) as work, \
             tc.tile_pool(name="pwork", bufs=3) as pwork, \
             tc.tile_pool(name="pj", bufs=2, space="PSUM") as pj, \
             tc.tile_pool(name="sc", bufs=2, space="PSUM") as sc, \
             tc.tile_pool(name="acc", bufs=2, space="PSUM") as acc:

            # ---- resident loads -------------------------------------------------
            wq_sb = per.tile([128, 16, QF], BF16, tag="wq")
            for _d in range(16):
                nc.sync.dma_start(out=wq_sb[:, _d, :], in_=wq_p[128 * _d:128 * (_d + 1), :])
            wkv_sb = per.tile([128, 16, 2 * HD], BF16, tag="wkv")
            for _d in range(16):
                nc.sync.dma_start(out=wkv_sb[:, _d, :], in_=wkv_p[128 * _d:128 * (_d + 1), :])
            cos_sb = per.tile([128, RT], BF16, tag="cos")
            ssin_sb = per.tile([128, RT], BF16, tag="ssin")
            tri_sb = per.tile([128, 512], BF16, tag="tri")
            nc.sync.dma_start(out=tri_sb[:], in_=tri_p[:, :])
            id_sb = per.tile([128, 128], BF16, tag="ident")
            nc.sync.dma_start(out=id_sb[:], in_=id_p[:, :])

            q2 = per.tile([128, B * NCH * 2 * CH], BF16, tag="qflat")  # p:{hA|hB}, cols:(b,c,f,pos)
            kt2 = per.tile([128, RT], BF16, tag="kt")                   # rows 0:64 = K, 64:128 = copy
            at_sb = [per.tile([128, RT], BF16, tag=f"at{i}", name=f"at{i}") for i in range(2)]
            vau = [per.tile([128, 2 * HD], BF16, tag=f"vau{i}", name=f"vau{i}") for i in range(RT // 128)]

            qv2 = q2[:, :].rearrange("p (b c f x) -> p b c f x", b=B, c=NCH, f=2, x=CH)

            qadd_insts = {}   # (b, cg) -> [4 add insts]
            kadd_insts = {}   # (b, cg) -> add inst
            vau_insts = {}    # chunk -> [copy, memset]
            norm_insts = {}   # (b, c) -> [4 mul insts]

            # ---- wo tensors + group emitter (interleaved with attention) -------
            outg = [dram.tile([D, 128 * (e - s0)], BF16, tag=f"og{g}", name=f"og{g}")
                    for g, (s0, e) in enumerate(GROUPS)]
            rsout = [dram.tile([D // M, 128 * (e - s0)], BF16, tag=f"rs{g}", name=f"rso{g}")
                     for g, (s0, e) in enumerate(GROUPS)]
            wo_sb = per.tile([128, 2, D], BF16, tag="wo")

            def emit_wo_group(g):
                s0, e = GROUPS[g]
                for base in range(s0, e, 4):
                    w = min(4, e - base)
                    ncols = 128 * w
                    coff = 128 * (base - s0)
                    for f in range(16):
                        ps = pj.tile([128, 512], F32, tag="pj", name="wops")
                        for cf in range(2):
                            nc.tensor.matmul(ps[:, 0:ncols], wo_sb[:, cf, 128 * f:128 * (f + 1)],
                                             at_sb[cf][:, 128 * base:128 * base + ncols],
                                             start=(cf == 0), stop=(cf == 1))
                        ob = pwork.tile([128, 512], BF16, tag="ob", name="ob")
                        if g >= 3:
                            nc.vector.tensor_copy(ob[:, 0:ncols], ps[:, 0:ncols])
                        else:
                            nc.any.tensor_copy(ob[:, 0:ncols], ps[:, 0:ncols])
                        nc.sync.dma_start(out=outg[g][128 * f:128 * (f + 1), coff:coff + ncols],
                                          in_=ob[:, 0:ncols])
                nc.gpsimd.collective_compute(
                    "ReduceScatter", mybir.AluOpType.add,
                    replica_groups=[list(range(M))],
                    ins=[outg[g].opt()], outs=[rsout[g].opt()])
                nc.sync.dma_start(out=out_p[:, 128 * s0:128 * e], in_=rsout[g][:])

            def emit_attn(b, c):
                o_ps = acc.tile([2 * HD, 512], F32, tag="acc", name="ops")
                blk = (b * NCH + c) * 256
                qA = q2[0:64, blk:blk + 256]
                qB = q2[64:128, blk:blk + 256]
                for j0 in range(0, c + 1, 2):
                    js = [j for j in (j0, j0 + 1) if j <= c]
                    s_ps = sc.tile([128, 1024], F32, tag="sc", name="sps")
                    for idx, j in enumerate(js):
                        kcol = b * S + CH * j
                        nc.tensor.matmul(s_ps[:, 256 * idx:256 * idx + 256],
                                         kt2[0:64, kcol:kcol + CH], qA,
                                         start=True, stop=True)
                        nc.tensor.matmul(s_ps[:, 512 + 256 * idx:512 + 256 * idx + 256],
                                         kt2[64:128, kcol:kcol + CH], qB,
                                         start=True, stop=True)
                    nw = 1024 if len(js) == 2 else 768
                    p_sb = pwork.tile([128, 1024], BF16, tag="p", name="psb")
                    nc.scalar.activation(p_sb[:, 0:nw], s_ps[:, 0:nw], Exp, scale=0.125)
                    if c in js:
                        idx = js.index(c)
                        for half in range(2):
                            off = 512 * half + 256 * idx
                            nc.vector.tensor_mul(p_sb[:, off:off + 256],
                                                 p_sb[:, off:off + 256], tri_sb[:, 0:256])
                    pv_view = p_sb[:, :].rearrange("p (w u v) -> p u w v", w=2, u=2, v=256)
                    for idx, j in enumerate(js):
                        nc.tensor.matmul(o_ps[:], vau[b * NCH + j][:],
                                         pv_view[:, idx, :, :],
                                         start=(j == 0), stop=(j == c))
                # o_ps col block i holds head hmap[i]; rows 64:128 = replicated denom
                hmap = (0, 2, 1, 3)
                den = pwork.tile([64, 512], F32, tag="den", name="den")
                nc.vector.tensor_copy(den[:], o_ps[HD:2 * HD, :])
                bc = pwork.tile([64, 512], F32, tag="bc", name="bct")
                nc.vector.reciprocal_approx_fast(bc[:], den[:])
                for i in range(4):
                    hh = hmap[i]
                    nc.vector.tensor_mul(
                        at_sb[hh // 2][64 * (hh % 2):64 * (hh % 2) + 64,
                                       b * S + CH * c: b * S + CH * (c + 1)],
                        o_ps[0:64, 128 * i:128 * (i + 1)],
                        bc[:, 128 * i:128 * (i + 1)])
                if b == 0 and c == 2:
                    nc.sync.dma_start(out=wo_sb[:], in_=wo_p[:, :].rearrange("(n p) f -> p n f", p=128))
                for g, (s0, e) in enumerate(GROUPS):
                    if b * NCH + c + 1 == e:
                        emit_wo_group(g)


            # ---- projections + RoPE, per 512-row slice -------------------------
            for rc in range(8):
                xr_t = xload.tile([128, 16, 512], BF16, tag=f"x{rc % 3}")
                if rc == 0:
                    # startup wave: spread across SP-HWDGE, ACT-HWDGE, gpsimd-SWDGE
                    for s in range(4):
                        d0 = 4 * s
                        nc.gpsimd.dma_start(
                            out=wkv_sb[:, d0:d0 + 4, :],
                            in_=wkv_p[128 * d0:128 * (d0 + 4), :].rearrange("(n p) f -> p n f", p=128))
                    def _xd(d, eng):
                        eng.dma_start(
                            out=xr_t[:, d, :],
                            in_=xt_p[128 * d:128 * (d + 1), 0:512]
                                .rearrange("(n p) f -> p n f", p=128))
                    _xd(0, nc.sync); _xd(1, nc.scalar)
                    nc.sync.dma_start(out=cos_sb[:, 0:512], in_=cos_p[:, 0:512])
                    nc.scalar.dma_start(out=ssin_sb[:, 0:512], in_=ssin_p[:, 0:512])
                    for d in range(2, 16):
                        _xd(d, nc.sync if d % 2 == 0 else nc.scalar)
                    for s in range(4):
                        d0 = 4 * s
                        (nc.sync if s % 2 == 0 else nc.scalar).dma_start(
                            out=wq_sb[:, d0:d0 + 4, :],
                            in_=wq_p[128 * d0:128 * (d0 + 4), :].rearrange("(n p) f -> p n f", p=128))
                    nc.sync.dma_start(out=tri_sb[:], in_=tri_p[:, :])
                    nc.scalar.dma_start(out=id_sb[:], in_=id_p[:, :])
                else:
                    for d in range(16):
                        (nc.sync if d % 2 == 0 else nc.scalar).dma_start(
                            out=xr_t[:, d, :],
                            in_=xt_p[128 * d:128 * (d + 1), 512 * rc:512 * (rc + 1)]
                                .rearrange("(n p) f -> p n f", p=128))
                    nc.sync.dma_start(out=cos_sb[:, 512 * rc:512 * (rc + 1)],
                                      in_=cos_p[:, 512 * rc:512 * (rc + 1)])
                    nc.scalar.dma_start(out=ssin_sb[:, 512 * rc:512 * (rc + 1)],
                                        in_=ssin_p[:, 512 * rc:512 * (rc + 1)])
                xr = [xr_t[:, d, :] for d in range(16)]
                cs = cos_sb[:, 512 * rc:512 * (rc + 1)]
                sn = ssin_sb[:, 512 * rc:512 * (rc + 1)]
                b, cg = rc // 4, rc % 4

                # Q: two 128-feature chunks (2 heads each)
                for f in range(2):
                    ps = pj.tile([128, 512], F32, tag="pj")
                    for d in range(16):
                        nc.tensor.matmul(ps[:], wq_sb[:, d, 128 * f:128 * (f + 1)], xr[d],
                                         start=(d == 0), stop=(d == 15))
                    t1 = work.tile([128, 512], F32, tag="t1")
                    nc.vector.tensor_mul(t1[:], ps[:], cs)
                    sw = work.tile([128, 512], F32, tag="sw")
                    for a, bq in ((0, 1), (1, 0), (2, 3), (3, 2)):
                        nc.any.tensor_copy(sw[32 * a:32 * (a + 1), :], ps[32 * bq:32 * (bq + 1), :])
                    t2 = work.tile([128, 512], F32, tag="t2")
                    nc.vector.tensor_mul(t2[:], sw[:], sn)
                    nc.vector.tensor_add(
                        qv2[:, b, 4 * cg:4 * (cg + 1), f, :],
                        t1[:, :].rearrange("p (a x) -> p a x", x=CH),
                        t2[:, :].rearrange("p (a x) -> p a x", x=CH))

                # K+V packed: one full-array matmul chain (k rows 0-63, v rows 64-127)
                ps = pj.tile([128, 512], F32, tag="pj")
                for d in range(16):
                    nc.tensor.matmul(ps[:], wkv_sb[:, d, :], xr[d],
                                     start=(d == 0), stop=(d == 15))
                t1 = work.tile([128, 512], F32, tag="t1")
                nc.vector.tensor_mul(t1[0:64, :], ps[0:64, :], cs[0:64, :])
                sw = work.tile([128, 512], F32, tag="sw")
                nc.any.tensor_copy(sw[0:32, :], ps[32:64, :])
                nc.any.tensor_copy(sw[32:64, :], ps[0:32, :])
                t2 = work.tile([128, 512], F32, tag="t2")
                nc.vector.tensor_mul(t2[0:64, :], sw[0:64, :], sn[0:64, :])
                nc.vector.tensor_add(kt2[0:64, 512 * rc:512 * (rc + 1)], t1[0:64, :], t2[0:64, :])
                nc.vector.tensor_add(kt2[64:128, 512 * rc:512 * (rc + 1)], t1[0:64, :], t2[0:64, :])

                vt = work.tile([64, 512], BF16, tag="vt")
                nc.vector.tensor_copy(vt[:], ps[64:128, :])
                for t in range(4):
                    tp = acc.tile([128, 64], F32, tag="acc")
                    nc.tensor.matmul(tp[:], vt[:, 128 * t:128 * (t + 1)], id_sb[0:64, 0:64],
                                     start=True, stop=True)
                    vtile = vau[4 * rc + t]
                    nc.vector.tensor_copy(vtile[:, 0:HD], tp[:])
                    nc.vector.memset(vtile[:, HD:2 * HD], 1.0)

                if rc > 0:
                    pb, pcg = (rc - 1) // 4, (rc - 1) % 4
                    for cc in range(4 * pcg, 4 * pcg + 4):
                        emit_attn(pb, cc)
            for cc in range(12, 16):
                emit_attn(1, cc)


    nc.compile()
    return nc


def _stage(x, cos, sin, wq, wk, wv, wo):
    xt = np.ascontiguousarray(x.reshape(RT, D).T).astype(bf16)
    cosT = cos.T.astype(np.float32)                      # [64, S]
    sinT = sin.T.astype(np.float32)
    cos2 = np.concatenate([cosT, cosT], axis=0)
    cos2 = np.concatenate([cos2, cos2], axis=1).astype(bf16)       # [128, RT]
    ssin1 = np.concatenate([-sinT[:HD // 2], sinT[HD // 2:]], axis=0)
    ssin2 = np.concatenate([ssin1, ssin1], axis=0)
    ssin2 = np.concatenate([ssin2, ssin2], axis=1).astype(bf16)
    tri4 = np.tile(np.triu(np.ones((CH, CH), np.float32)), (1, 4)).astype(bf16)
    ident = np.eye(128, dtype=np.float32).astype(bf16)

    in_maps = []
    for m in range(M):
        in_maps.append({
            "xt": xt,
            "cos2": cos2,
            "ssin2": ssin2,
            "wqs": np.ascontiguousarray(wq[QF * m:QF * (m + 1), :].T).astype(bf16),
            "wkvs": np.ascontiguousarray(np.concatenate(
                [wk[HD * m:HD * (m + 1), :].T, wv[HD * m:HD * (m + 1), :].T], axis=1)).astype(bf16),
            "wos": np.ascontiguousarray(wo[:, QF * m:QF * (m + 1)].T).astype(bf16),
            "tri4": tri4,
            "ident": ident,
        })
    return in_maps


def kernel(x, cos, sin, wq, wk, wv, wo):
    from concourse.bass_utils import run_bass_kernel_spmd

    if "nc" not in _CACHE:
        _CACHE["nc"] = _build_nc()
    nc = _CACHE["nc"]

    in_maps = _stage(x, cos, sin, wq, wk, wv, wo)
    res = run_bass_kernel_spmd(nc, in_maps, list(range(M)), **RUN_OPTS)
    LAST_RESULT[0] = res

    outT = np.concatenate([np.asarray(res.results[m]["out"]).astype(np.float32)
                           for m in range(M)], axis=0)       # [D, RT]
    return np.ascontiguousarray(outT.T).reshape(B, S, D)

